# revision 8
# baseline (speedup 1.0000x reference)
"""Trainium2 Bass kernel for nn_BondWeight (symmetric edge-weight scatter).

Problem: out[b, src[b,e]+1, dst[b,e]+1] = w[b,e] and
         out[b, dst[b,e]+1, src[b,e]+1] = w[b,e]  (set semantics, XLA-CPU
         last-write-wins order: full scatter-1 pass then scatter-2 pass),
         where w = weights[bond_type], out is [1024, 256, 256] f32 zeros.

Strategy (8 NeuronCores, data-parallel over batch, 128 batches/core):
  Host: gather weights, compute write positions, dedup duplicate positions
        keeping only the final writer (reproduces XLA-CPU set semantics),
        then pack per (batch, partition) scatter lists. f32 values are
        split into lo/hi int16 halves (bit-exact).
  Device (per core): for each batch, GPSIMD `local_scatter` builds the
        zeroed + scattered [128 x 1024] int16 tile (= the batch's 256x256
        f32 matrix, rows 2p,2p+1 on partition p) in Q7-local RAM and
        streams it to SBUF; the tile is then DMAed contiguously to the
        output. Double-buffered so GPSIMD and DMA overlap.
"""

import numpy as np

B, E, T, N = 1024, 512, 8, 256
M = 8                      # cores
BL = B // M                # 128 batches per core
NN = N * N                 # 65536
TOT = BL * NN              # 8388608 f32 elements per core (32 MB)
PARTS = 128                # SBUF partitions; partition p holds rows 2p,2p+1
ELEMS = 2 * N * 2          # 1024 int16 elements per partition per batch
NBUF = 4                   # double-buffering depth for dst tiles

_nc_cache = {}


def _prepare_scatter(weights, bond_src, bond_dst, bond_type):
    """Returns (idx, dat, niw): int16 arrays [M, PARTS, BL*niw].

    For core m, partition p, batch b, slots [b*niw, (b+1)*niw): pairs of
    (int16 index into the partition's 1024-int16 batch tile, int16 half of
    the f32 value). Padded slots have idx == -1 (ignored by local_scatter).
    """
    w = np.ascontiguousarray(weights, dtype=np.float32)[np.asarray(bond_type)]
    s = np.asarray(bond_src, dtype=np.int64) + 1
    d = np.asarray(bond_dst, dtype=np.int64) + 1
    bb = np.arange(B, dtype=np.int64)[:, None]
    # Flat (batch, row, col) key for both scatter passes, [B, 2E]
    key = np.concatenate([bb * NN + s * N + d, bb * NN + d * N + s],
                         axis=1).ravel()
    # Write-order within a batch: scatter1 edges 0..E-1, then scatter2
    order = np.tile(np.arange(2 * E, dtype=np.int64), B)
    vals = np.concatenate([w, w], axis=1).ravel()

    sortidx = np.lexsort((order, key))
    ksort = key[sortidx]
    is_last = np.empty(len(ksort), dtype=bool)
    is_last[:-1] = ksort[1:] != ksort[:-1]
    is_last[-1] = True
    sel = sortidx[is_last]            # final writer of each position
    fkey = key[sel]                   # ascending
    fval = vals[sel]

    gb = fkey // NN                   # global batch
    q = fkey % NN
    r = q // N                        # row
    c = q % N                         # col
    m = gb // BL                      # core
    b = gb % BL                       # batch within core
    p = r // 2                        # partition
    qq = (r % 2) * N + c              # f32 position within partition tile

    # slot within each (core, batch, partition) group; fkey asc => groups
    # are contiguous (row-major, rows 2p,2p+1 adjacent)
    grp = (m * BL + b) * PARTS + p
    n_ent = len(grp)
    new_grp = np.empty(n_ent, dtype=bool)
    new_grp[0] = True
    new_grp[1:] = grp[1:] != grp[:-1]
    gstart = np.maximum.accumulate(
        np.where(new_grp, np.arange(n_ent), 0))
    cc = np.arange(n_ent) - gstart    # cumcount within group

    niw = 2 * (int(cc.max()) + 1)
    if niw % 2:
        niw += 1

    bits = fval.view(np.uint32).astype(np.int64)
    lo = (bits & 0xFFFF).astype(np.uint16).view(np.int16)
    hi = ((bits >> 16) & 0xFFFF).astype(np.uint16).view(np.int16)

    idx = np.full((M, PARTS, BL * niw), -1, dtype=np.int16)
    dat = np.zeros((M, PARTS, BL * niw), dtype=np.int16)
    col = b * niw + 2 * cc
    idx[m, p, col] = (2 * qq).astype(np.int16)
    idx[m, p, col + 1] = (2 * qq + 1).astype(np.int16)
    dat[m, p, col] = lo
    dat[m, p, col + 1] = hi
    return idx, dat, niw


def _build_nc(niw):
    import concourse.bass as bass
    import concourse.mybir as mybir
    from concourse import library_config

    nc = bass.Bass("TRN2", target_bir_lowering=False)
    idx_t = nc.dram_tensor("lsidx", [PARTS, BL * niw], mybir.dt.int16,
                           kind="ExternalInput")
    dat_t = nc.dram_tensor("lsdat", [PARTS, BL * niw], mybir.dt.int16,
                           kind="ExternalInput")
    # int16 view of the [BL, 256, 256] f32 output: batch b, partition p ->
    # int16 row b*PARTS + p (f32 rows 2p, 2p+1 of batch b)
    out_t = nc.dram_tensor("out", [BL * PARTS, ELEMS], mybir.dt.int16,
                           kind="ExternalOutput")

    with (
        nc.sbuf_tensor("idx_sb", [PARTS, BL * niw], mybir.dt.int16) as idx_sb,
        nc.sbuf_tensor("dat_sb", [PARTS, BL * niw], mybir.dt.int16) as dat_sb,
        nc.sbuf_tensor("dst_sb", [PARTS, NBUF * ELEMS], mybir.dt.int16) as dst_sb,
        nc.semaphore("in_sem") as in_sem,
        nc.semaphore("ls_sem") as ls_sem,
        nc.semaphore("dma_sem") as dma_sem,
        nc.Block() as block,
    ):
        @block.gpsimd
        def _(gpsimd):
            gpsimd.load_library(library_config.local_scatter)
            gpsimd.wait_ge(in_sem, 32)
            for b in range(BL):
                if b >= NBUF:
                    gpsimd.wait_ge(dma_sem, 16 * (b - NBUF + 1))
                k = b % NBUF
                gpsimd.local_scatter(
                    out_ap=dst_sb[:, k * ELEMS:(k + 1) * ELEMS],
                    data_ap=dat_sb[:, b * niw:(b + 1) * niw],
                    idxs_ap=idx_sb[:, b * niw:(b + 1) * niw],
                    channels=PARTS,
                    num_elems=ELEMS,
                    num_idxs=niw,
                ).then_inc(ls_sem, 1)

        @block.sync
        def _(sync):
            sync.dma_start(idx_sb[:], idx_t[:]).then_inc(in_sem, 16)
            sync.dma_start(dat_sb[:], dat_t[:]).then_inc(in_sem, 16)
            for b in range(BL):
                sync.wait_ge(ls_sem, b + 1)
                k = b % NBUF
                out_ap = bass.AP(out_t, b * PARTS * ELEMS,
                                 [[ELEMS, PARTS], [1, ELEMS]])
                sync.dma_start(out_ap,
                               dst_sb[:, k * ELEMS:(k + 1) * ELEMS]) \
                    .then_inc(dma_sem, 16)
            sync.wait_ge(dma_sem, 16 * BL)

    # Raw Bass skips Bacc's codegen pass that fills in .instr bytes for
    # extended-inst InstISA subclasses (InstLocalScatter) -> "ISA wrong
    # length" in the NEFF compiler without this.
    from concourse.library_overlay import lower_extended_insts
    lower_extended_insts(nc)
    return nc


def _get_nc(niw):
    if niw not in _nc_cache:
        _nc_cache[niw] = _build_nc(niw)
    return _nc_cache[niw]


def run_with_stats(inputs, trace=False):
    """Run the kernel; returns (output [B,N,N] f32, exec_time_ns or None)."""
    from concourse.bass_utils import run_bass_kernel_spmd

    idx, dat, niw = _prepare_scatter(inputs["weights"], inputs["bond_src"],
                                     inputs["bond_dst"], inputs["bond_type"])
    nc = _get_nc(niw)
    in_maps = [{"lsidx": np.ascontiguousarray(idx[m]),
                "lsdat": np.ascontiguousarray(dat[m])} for m in range(M)]
    res = run_bass_kernel_spmd(nc, in_maps, core_ids=list(range(M)),
                               trace=trace)
    out = np.empty((B, N, N), dtype=np.float32)
    for m in range(M):
        o = res.results[m]["out"]            # int16 [BL*PARTS, ELEMS]
        out[m * BL:(m + 1) * BL] = o.reshape(BL, PARTS * ELEMS) \
            .view(np.float32).reshape(BL, N, N)
    return out, res.exec_time_ns


def kernel(weights, bond_src, bond_dst, bond_type, num_nodes):
    assert int(num_nodes) == N
    out, _ = run_with_stats({
        "weights": np.asarray(weights),
        "bond_src": np.asarray(bond_src),
        "bond_dst": np.asarray(bond_dst),
        "bond_type": np.asarray(bond_type),
    })
    return out
